# revision 9
# baseline (speedup 1.0000x reference)
"""Trainium2 Bass kernel for nn_COINBlock (rope -> relu -> +bias -> tanh -> GroupNorm).

Reference computation (B=2, T=4096, I=C=2048, fp32):
    Q   = relu(rope(X @ W_Q))            # rope on first 64 channels, interleaved pairs
    H   = tanh(Q + hh_b)
    out = GroupNorm32(H) (per row, 32 groups x 64 ch), then
    out = swapaxes(out, -2, -1).reshape(B, T, C)   # pure layout permutation
    returns (out, S_n)  with S_n passed through

Sharding: data-parallel over the 8192 flattened rows -> 1024 rows/core on 8 cores.
Each core computes a (1024x2048) @ (2048x2048) fp32 matmul (float32r PE mode)
plus the elementwise/GroupNorm epilogue, writing its rows in natural layout.
The final transpose-reshape is a pure element permutation done while unsharding.
"""

import sys

for _p in ("/opt/trn_rl_repo",):
    if _p not in sys.path:
        sys.path.append(_p)

import numpy as np

import concourse.bass as bass
import concourse.tile as tile
from concourse import bacc, mybir
from concourse.bass_utils import run_bass_kernel_spmd

F32 = mybir.dt.float32
F32R = mybir.dt.float32r

B, T, I, C = 2, 4096, 2048, 2048
N_CORES = 8
ROWS = (B * T) // N_CORES          # 1024 rows per core
P = 128                            # partitions
KT = I // P                        # 16 k-tiles
NB = 4                             # 4 psum banks of 512 output channels
NS = C // NB                       # 512
GN_GROUPS, GD = 32, C // 32        # 32 groups x 64 channels
ROPE_DIM = 64
GN_EPS = 1e-5


def emit_kernel(ctx, tc, aps, mt, apply_gnwb, reps):
    """Emit the per-core Tile kernel. `mt` = number of 128-row m-tiles."""
    nc = tc.nc
    xt_ap, w_ap, cos_ap, sin_ap, hb_ap, gw_ap, gb_ap, out_ap = aps
    AF = mybir.ActivationFunctionType
    OP = mybir.AluOpType

    wpool = ctx.enter_context(tc.tile_pool(name="w", bufs=1))
    singles = ctx.enter_context(tc.tile_pool(name="singles", bufs=1))
    xpool = ctx.enter_context(tc.tile_pool(name="x", bufs=2))
    hpool = ctx.enter_context(tc.tile_pool(name="h", bufs=2))
    pspool = ctx.enter_context(tc.tile_pool(name="ps", bufs=4, space="PSUM"))
    rqpool = ctx.enter_context(tc.tile_pool(name="rq", bufs=2))
    stpool = ctx.enter_context(tc.tile_pool(name="st", bufs=2))
    smpool = ctx.enter_context(tc.tile_pool(name="sm", bufs=4))

    # One-time loads: rope tables, broadcast bias (and gn w/b if needed).
    cos_t = singles.tile([P, mt, 32], F32)
    nc.sync.dma_start(out=cos_t[:], in_=cos_ap[:])
    sin_t = singles.tile([P, mt, 32], F32)
    nc.sync.dma_start(out=sin_t[:], in_=sin_ap[:])
    b_t = singles.tile([P, C], F32)
    nc.gpsimd.dma_start(out=b_t[:], in_=hb_ap[:].partition_broadcast(P))
    eps_t = singles.tile([P, 1], F32)
    nc.vector.memset(eps_t[:], GN_EPS)
    if apply_gnwb:
        gw_t = singles.tile([P, C], F32)
        nc.gpsimd.dma_start(out=gw_t[:], in_=gw_ap[:].partition_broadcast(P))
        gb_t = singles.tile([P, C], F32)
        nc.gpsimd.dma_start(out=gb_t[:], in_=gb_ap[:].partition_broadcast(P))

    for _rep in range(reps):
        wt = []
        for k in range(KT):
            wk = wpool.tile([P, C], F32R, tag=f"w{k % 16}")
            nc.sync.dma_start(out=wk[:], in_=w_ap[bass.ts(k, P), :])
            wt.append(wk)

        for m in range(mt):
            xt_m = xpool.tile([P, KT * P], F32R)
            nc.sync.dma_start(out=xt_m[:], in_=xt_ap[m, :, :])

            h_m = hpool.tile([P, C], F32)
            sums = stpool.tile([P, GN_GROUPS], F32, tag="sums")
            sumsq = stpool.tile([P, GN_GROUPS], F32, tag="sumsq")

            for n in range(NB):
                ps = pspool.tile([P, NS], F32)
                for k in range(KT):
                    nc.tensor.matmul(
                        ps[:],
                        lhsT=xt_m[:, bass.ts(k, P)],
                        rhs=wt[k][:, bass.ts(n, NS)],
                        start=(k == 0),
                        stop=(k == KT - 1),
                    )
                if n == 0:
                    # rope on channels 0:64 (interleaved pairs), before relu.
                    psv = ps[:, 0:ROPE_DIM].rearrange("p (q two) -> p q two", two=2)
                    pe, po = psv[:, :, 0], psv[:, :, 1]
                    cm, sm_ = cos_t[:, m, :], sin_t[:, m, :]
                    rq = rqpool.tile([P, ROPE_DIM], F32)
                    rqv = rq[:].rearrange("p (q two) -> p q two", two=2)
                    t1 = smpool.tile([P, 32], F32, tag="ropetmp")
                    nc.vector.tensor_mul(t1[:], po, sm_)        # odd*sin
                    t2 = smpool.tile([P, 32], F32, tag="ropetmp")
                    nc.vector.tensor_mul(t2[:], pe, cm)         # even*cos
                    nc.vector.tensor_sub(rqv[:, :, 0], t2[:], t1[:])
                    t3 = smpool.tile([P, 32], F32, tag="ropetmp")
                    nc.vector.tensor_mul(t3[:], po, cm)         # odd*cos
                    t4 = smpool.tile([P, 32], F32, tag="ropetmp")
                    nc.vector.tensor_mul(t4[:], pe, sm_)        # even*sin
                    nc.vector.tensor_add(rqv[:, :, 1], t3[:], t4[:])
                    # H = relu(q) + b, fused: (q max 0) add b
                    nc.vector.scalar_tensor_tensor(
                        out=h_m[:, 0:ROPE_DIM], in0=rq[:], scalar=0.0,
                        in1=b_t[:, 0:ROPE_DIM], op0=OP.max, op1=OP.add)
                    nc.vector.scalar_tensor_tensor(
                        out=h_m[:, ROPE_DIM:NS], in0=ps[:, ROPE_DIM:NS], scalar=0.0,
                        in1=b_t[:, ROPE_DIM:NS], op0=OP.max, op1=OP.add)
                else:
                    nc.vector.scalar_tensor_tensor(
                        out=h_m[:, bass.ts(n, NS)], in0=ps[:], scalar=0.0,
                        in1=b_t[:, bass.ts(n, NS)], op0=OP.max, op1=OP.add)
                hs = h_m[:, bass.ts(n, NS)]
                nc.scalar.activation(out=hs, in_=hs, func=AF.Tanh)
                # per-group (64ch) stats: sum and sum of squares
                nc.vector.tensor_reduce(
                    out=sums[:, bass.ts(n, 8)],
                    in_=hs.rearrange("p (g d) -> p g d", d=GD),
                    axis=mybir.AxisListType.X, op=OP.add)
                sq = smpool.tile([P, NS], F32, tag="sq")
                nc.scalar.square(out=sq[:], in_=hs)
                nc.vector.tensor_reduce(
                    out=sumsq[:, bass.ts(n, 8)],
                    in_=sq[:].rearrange("p (g d) -> p g d", d=GD),
                    axis=mybir.AxisListType.X, op=OP.add)

            # mean/var per group: var = sumsq/64 - (sums/64)^2
            msq = smpool.tile([P, GN_GROUPS], F32, tag="msq")
            nc.vector.scalar_tensor_tensor(
                out=msq[:], in0=sums[:], scalar=1.0 / (GD * GD), in1=sums[:],
                op0=OP.mult, op1=OP.mult)
            var = smpool.tile([P, GN_GROUPS], F32, tag="var")
            nc.vector.scalar_tensor_tensor(
                out=var[:], in0=sumsq[:], scalar=1.0 / GD, in1=msq[:],
                op0=OP.mult, op1=OP.subtract)
            std = smpool.tile([P, GN_GROUPS], F32, tag="std")
            nc.scalar.activation(out=std[:], in_=var[:], func=AF.Sqrt,
                                 bias=eps_t[:])
            rstd = smpool.tile([P, GN_GROUPS], F32, tag="rstd")
            nc.vector.reciprocal(out=rstd[:], in_=std[:])
            ms = smpool.tile([P, GN_GROUPS], F32, tag="ms")
            nc.vector.scalar_tensor_tensor(
                out=ms[:], in0=sums[:], scalar=1.0 / GD, in1=rstd[:],
                op0=OP.mult, op1=OP.mult)

            # apply: out = H*rstd - mean*rstd, broadcast per group of 64.
            hg = h_m[:].rearrange("p (g d) -> p g d", d=GD)
            rstd_b = rstd[:].unsqueeze(2).broadcast_to((P, GN_GROUPS, GD))
            ms_b = ms[:].unsqueeze(2).broadcast_to((P, GN_GROUPS, GD))
            nc.vector.tensor_mul(hg, hg, rstd_b)
            nc.gpsimd.tensor_sub(hg, hg, ms_b)
            if apply_gnwb:
                nc.vector.tensor_mul(h_m[:], h_m[:], gw_t[:])
                nc.gpsimd.tensor_add(h_m[:], h_m[:], gb_t[:])

            nc.sync.dma_start(out=out_ap[bass.ts(m, P), :], in_=h_m[:])


def build(rows=ROWS, reps=1, apply_gnwb=False):
    """Build and compile the Bass module for one core (SPMD across 8)."""
    assert rows % P == 0
    mt = rows // P
    nc = bacc.Bacc("TRN2", target_bir_lowering=False, debug=False,
                   enable_asserts=False, num_devices=N_CORES)
    xt_ap = nc.dram_tensor("xt", [mt, P, KT * P], F32R, kind="ExternalInput").ap()
    w_ap = nc.dram_tensor("w", [I, C], F32R, kind="ExternalInput").ap()
    cos_ap = nc.dram_tensor("cos", [P, mt, 32], F32, kind="ExternalInput").ap()
    sin_ap = nc.dram_tensor("sin", [P, mt, 32], F32, kind="ExternalInput").ap()
    hb_ap = nc.dram_tensor("hb", [1, C], F32, kind="ExternalInput").ap()
    if apply_gnwb:
        gw_ap = nc.dram_tensor("gw", [1, C], F32, kind="ExternalInput").ap()
        gb_ap = nc.dram_tensor("gb", [1, C], F32, kind="ExternalInput").ap()
    else:
        gw_ap = gb_ap = None
    out_ap = nc.dram_tensor("h", [rows, C], F32, kind="ExternalOutput").ap()

    from contextlib import ExitStack
    with tile.TileContext(nc) as tc, ExitStack() as ctx:
        emit_kernel(ctx, tc,
                    (xt_ap, w_ap, cos_ap, sin_ap, hb_ap, gw_ap, gb_ap, out_ap),
                    mt, apply_gnwb, reps)
    nc.compile()
    return nc


def make_core_inputs(Xf, W, hh_b, core, rows=ROWS, gn_weight=None, gn_bias=None):
    """Build the input map for one core. Xf = (8192, 2048) flattened rows."""
    mt = rows // P
    r0 = rows * core
    shard = Xf[r0:r0 + rows]
    xt = np.ascontiguousarray(
        shard.reshape(mt, P, KT, P).transpose(0, 3, 2, 1)).reshape(mt, P, KT * P)
    t0 = r0 % T
    t = np.arange(t0, t0 + rows, dtype=np.float32)
    inv = (1.0 / (10000.0 ** (np.arange(0, ROPE_DIM, 2, dtype=np.float32)
                              / ROPE_DIM))).astype(np.float32)
    fr = t[:, None] * inv[None, :]                       # (rows, 32)
    cos = np.ascontiguousarray(
        np.cos(fr).astype(np.float32).reshape(mt, P, 32).transpose(1, 0, 2))
    sin = np.ascontiguousarray(
        np.sin(fr).astype(np.float32).reshape(mt, P, 32).transpose(1, 0, 2))
    m = {"xt": xt, "w": W, "cos": cos, "sin": sin, "hb": hh_b.reshape(1, C)}
    if gn_weight is not None:
        m["gw"] = gn_weight.reshape(1, C)
        m["gb"] = gn_bias.reshape(1, C)
    return m


_NC_CACHE = {}


def kernel(X, S_n, W_Q, hh_b, gn_weight, gn_bias):
    X = np.ascontiguousarray(np.asarray(X, dtype=np.float32))
    S_n = np.asarray(S_n, dtype=np.float32)
    W_Q = np.ascontiguousarray(np.asarray(W_Q, dtype=np.float32))
    hh_b = np.ascontiguousarray(np.asarray(hh_b, dtype=np.float32))
    gn_weight = np.ascontiguousarray(np.asarray(gn_weight, dtype=np.float32))
    gn_bias = np.ascontiguousarray(np.asarray(gn_bias, dtype=np.float32))

    apply_gnwb = not (np.all(gn_weight == 1.0) and np.all(gn_bias == 0.0))
    key = ("main", apply_gnwb)
    if key not in _NC_CACHE:
        _NC_CACHE[key] = build(rows=ROWS, reps=1, apply_gnwb=apply_gnwb)
    nc = _NC_CACHE[key]

    Xf = X.reshape(B * T, I)
    in_maps = [
        make_core_inputs(Xf, W_Q, hh_b, c, ROWS,
                         gn_weight if apply_gnwb else None,
                         gn_bias if apply_gnwb else None)
        for c in range(N_CORES)
    ]
    res = run_bass_kernel_spmd(nc, in_maps, core_ids=list(range(N_CORES)))
    Y = np.concatenate([res.results[c]["h"] for c in range(N_CORES)], axis=0)
    out = np.swapaxes(Y.reshape(B, T, C), 1, 2).reshape(B, T, C)
    return out, S_n
